# revision 38
# baseline (speedup 1.0000x reference)
"""Distributed GATv2 GNN kernel for trn2 (8 NeuronCores).

Sharding: nodes are degree-sorted and striped across 8 cores (graph
partition by destination). Each core processes the incoming-edge segments
of its nodes in blocks of 128 dst lanes with a uniform per-block padded
degree schedule K_sched. Per layer:
  - every core computes the full xl = h @ Wl table (fp16, node-major,
    DRAM) with fp16 matmuls,
  - per dst block: dma_gather of fp16 xl[src] rows, GATv2 attention +
    softmax in fp16/fp32 on DVE/ACT, alpha-weighted sum accumulated by a
    single fp32 TensorReduce over the degree axis, PE transpose into a
    feature-major fp16 local slab,
  - AllGather of the fp16 pre-BN slab; gathered slabs stay resident in
    SBUF: global BN stats + ReLU (+ residual into fp32 h) computed
    redundantly on every core, with a fp16 shadow h16 feeding the next
    layer's matmuls.
Weights are replicated. Host-staged tensors are deduplicated (idx 16-row
stream, att rows, single-copy Wl/Wr halves, 17-feature x slices) and
re-expanded on device, cutting host->device staging from 41.5 MB to
17.1 MB across the 8 cores. imp/pol are packed into one [2, NPAD] output
tensor so the result fetch is a single sharded-array pull.
"""
import numpy as np
from dataclasses import dataclass

import concourse.bass as bass
import concourse.bacc as bacc
import concourse.tile as tile
import concourse.mybir as mybir

AF = mybir.ActivationFunctionType
ALU = mybir.AluOpType
FP32 = mybir.dt.float32
F16 = mybir.dt.float16
I16 = mybir.dt.int16

SLOPE = 0.2
EPS = 1e-5
NEG = -1.0e30
NEG16 = -60000.0
F16NP = np.float16


@dataclass
class Cfg:
    ncores: int = 8
    blocks: int = 20               # dst blocks per core
    real_per_core: int = 2500      # real nodes per core
    nlayers: int = 20              # total GAT layers (first + mids + final)
    K_sched: tuple = ()            # per-block padded degree (shared by cores)
    f_in: int = 17
    stage: int = 4
    bstage: int = 7
    gather_layers: tuple = tuple(range(32))
    dump_layer: int = -1
    gp_bufs: int = 4
    sp_bufs: int = 2
    sm_bufs: int = 4
    mmp_bufs: int = 4
    trp_bufs: int = 2
    oc_bufs: int = 2
    st_bufs: int = 6
    alpha_on_dve: bool = True
    alpha_dve_kmax: int = 0        # blocks with K <= this use DVE for alpha
    dma_spread: bool = False       # alternate bulk DMAs across SP/ACT queues
    stg_mode: int = 0              # 0=ACT, 1=DVE, 2=alternate ACT/DVE
    h16_on_pool: bool = True       # h16 fp16 shadow copies on gpsimd
    h16_on_dve: bool = False       # h16 fp16 shadow copies on DVE
    red_on_pool: bool = False      # weighted-sum reduce on gpsimd
    lg_on_pool: bool = False       # logit reduce on gpsimd
    satt_on_pool: bool = False     # s*att multiply on gpsimd
    hadd_on_pool: bool = False     # h_fold residual adds on gpsimd
    red_xy: bool = False           # fuse weighted-sum reduce + head sum (XY)
    slab_ring: int = 2             # 0 = persistent slab16, else ring bufs

    @property
    def slots(self):
        return self.blocks * 128

    @property
    def npad(self):
        return self.ncores * self.slots

    @property
    def half2(self):
        return self.npad // 2

    @property
    def sumk(self):
        return int(sum(self.K_sched))

    def width(self, l):           # H*D of layer l
        return 128 if l < self.nlayers - 1 else 64

    def kcontract(self, l):       # matmul contraction dim
        return 32 if l == 0 else 64


def build_kernel(cfg: Cfg):
    NC = cfg.ncores
    SLOTS = cfg.slots
    NPAD = cfg.npad
    HALF2 = cfg.half2
    L = cfg.nlayers
    SUMK = cfg.sumk
    KMAX = int(max(cfg.K_sched))
    N_REAL = NC * cfg.real_per_core
    X = mybir.AxisListType.X

    nc = bacc.Bacc("TRN2", target_bir_lowering=False, debug=False, num_devices=NC)

    # ---------------- DRAM I/O ----------------
    # host stages deduplicated tensors; the kernel re-expands on device
    idx_d = nc.dram_tensor("idx", [16, 8 * SUMK], I16, kind="ExternalInput")
    mask_d = nc.dram_tensor("mask", [128, SUMK], F16, kind="ExternalInput")
    xTf_d = nc.dram_tensor("xTf", [34, HALF2], F16, kind="ExternalInput")
    xloc_d = nc.dram_tensor("xloc", [17, SLOTS], FP32, kind="ExternalInput")
    Wl_d = nc.dram_tensor("Wl", [L, 64, 128], F16, kind="ExternalInput")
    Wr_d = nc.dram_tensor("Wr", [L, 64, 128], FP32, kind="ExternalInput")
    attR_d = nc.dram_tensor("attR", [L, 1, 128], FP32, kind="ExternalInput")
    xrb_d = nc.dram_tensor("xrb", [L, 128], FP32, kind="ExternalInput")
    beff_d = nc.dram_tensor("beff", [L, 64], FP32, kind="ExternalInput")
    g_d = nc.dram_tensor("gbn", [L, 64], FP32, kind="ExternalInput")
    be_d = nc.dram_tensor("bebn", [L, 64], FP32, kind="ExternalInput")
    ident_d = nc.dram_tensor("ident", [128, 128], FP32, kind="ExternalInput")
    headW_d = nc.dram_tensor("headW", [128, 2], FP32, kind="ExternalInput")
    headb_d = nc.dram_tensor("headb", [2, 1], FP32, kind="ExternalInput")

    out_d = nc.dram_tensor("out", [2, NPAD], FP32, kind="ExternalOutput")
    dbg_d = None
    if cfg.dump_layer >= 0:
        dbg_d = nc.dram_tensor("out_dbg", [128, HALF2], FP32, kind="ExternalOutput")

    with tile.TileContext(nc) as tc:
        with (
            tc.tile_pool(name="persist", bufs=1) as P,
            tc.tile_pool(name="wload", bufs=2) as WP,
            tc.tile_pool(name="gpool", bufs=cfg.gp_bufs) as GP,
            tc.tile_pool(name="spool", bufs=cfg.sp_bufs) as SP,
            tc.tile_pool(name="small", bufs=cfg.sm_bufs) as SM,
            tc.tile_pool(name="stage", bufs=cfg.st_bufs) as ST,
            tc.tile_pool(name="ochunk", bufs=cfg.oc_bufs) as OC,
            tc.tile_pool(name="xstream", bufs=2) as XS,
            tc.tile_pool(name="mm_ps", bufs=cfg.mmp_bufs, space="PSUM") as MMP,
            tc.tile_pool(name="xr_ps", bufs=2, space="PSUM") as XRP,
            tc.tile_pool(name="tr_ps", bufs=cfg.trp_bufs, space="PSUM") as TRP,
            tc.tile_pool(name="dram", bufs=2, space="DRAM") as DP,
        ):
            # ---------------- persistent SBUF ----------------
            h_fold = P.tile([128, HALF2], FP32, tag="h_fold")
            h16 = P.tile([128, HALF2], F16, tag="h16")
            h_loc = P.tile([64, SLOTS], FP32, tag="h_loc")
            idx_sb = P.tile([128, 8 * SUMK], I16, tag="idx_sb")
            mask_sb = P.tile([128, SUMK], F16, tag="mask_sb")
            ones_sb = P.tile([1, 128], FP32, tag="ones_sb")
            ident_sb = P.tile([128, 128], FP32, tag="ident_sb")
            o_slab = P.tile([64, SLOTS], F16, tag="o_slab")
            slab16 = None
            if not cfg.slab_ring:
                slab16 = P.tile([128, 4, SLOTS], F16, tag="slab16")
            xloc_sb = P.tile([17, SLOTS], FP32, tag="xloc_sb")

            # expand host-deduplicated inputs on device
            for i in range(8):
                nc.sync.dma_start(idx_sb[16 * i : 16 * i + 16, :], idx_d[:, :])
            nc.sync.dma_start(mask_sb[:], mask_d[:, :])
            nc.sync.dma_start(ident_sb[:], ident_d[:, :])
            nc.sync.dma_start(xloc_sb[:], xloc_d[:, :])
            nc.vector.memset(ones_sb[:], 1.0)
            if cfg.stage < 4:
                nc.vector.memset(h_fold[:], 0.0)
                nc.vector.memset(h16[:], 0.0)
                nc.vector.memset(h_loc[:], 0.0)
                nc.vector.memset(o_slab[:], 0.0)

            koff = [0]
            for K in cfg.K_sched:
                koff.append(koff[-1] + int(K))

            for l in range(L):
                w = cfg.width(l)       # H*D of this layer
                w2 = w // 2            # per-head width = output width
                KC = cfg.kcontract(l)  # matmul contraction
                last = l == L - 1

                # -------- per-layer weight loads (duplicate halves on device) --
                Wl_sb = WP.tile([128, 128], F16, tag="Wl_sb")
                nc.sync.dma_start(
                    Wl_sb[0:KC, :], Wl_d.ap()[l : l + 1, 0:KC].squeeze(0)
                )
                nc.sync.dma_start(
                    Wl_sb[KC : 2 * KC, :], Wl_d.ap()[l : l + 1, 0:KC].squeeze(0)
                )
                Wr_sb = WP.tile([128, 128], FP32, tag="Wr_sb")
                nc.sync.dma_start(
                    Wr_sb[0:KC, :], Wr_d.ap()[l : l + 1, 0:KC].squeeze(0)
                )
                nc.sync.dma_start(
                    Wr_sb[KC : 2 * KC, :], Wr_d.ap()[l : l + 1, 0:KC].squeeze(0)
                )
                # broadcast att row to all 128 partitions via rank-1 matmul
                attRow = WP.tile([1, 128], FP32, tag="attRow")
                nc.sync.dma_start(
                    attRow[:], attR_d.ap()[l : l + 1, :, :].squeeze(0)
                )
                attb_ps = XRP.tile([128, 128], FP32, tag="xr")
                nc.tensor.matmul(
                    attb_ps[:, 0:w], ones_sb[:], attRow[:, 0:w],
                    start=True, stop=True,
                )
                attR_sb = WP.tile([128, 128], F16, tag="attR_sb")
                nc.scalar.copy(attR_sb[:, 0:w], attb_ps[:, 0:w])
                xrb_sb = WP.tile([1, 128], FP32, tag="xrb_sb")
                nc.sync.dma_start(xrb_sb[:], xrb_d.ap()[l : l + 1, :])
                beff_sb = WP.tile([64, 1], FP32, tag="beff_sb")
                nc.sync.dma_start(beff_sb[:], beff_d.ap()[l : l + 1, :].rearrange("o f -> f o"))
                g_sb = WP.tile([64, 1], FP32, tag="g_sb")
                nc.sync.dma_start(g_sb[:], g_d.ap()[l : l + 1, :].rearrange("o f -> f o"))
                be_sb = WP.tile([64, 1], FP32, tag="be_sb")
                nc.sync.dma_start(be_sb[:], be_d.ap()[l : l + 1, :].rearrange("o f -> f o"))

                # -------- xl table: [NPAD, 128-pitch] fp16 in DRAM --------
                xl_tab = DP.tile([NPAD, 128], F16, tag="xl_tab")
                n_groups = HALF2 // 512
                # layer 0 contracts over the 17 real input features only
                KR = 17 if l == 0 else KC
                for g in range(n_groups):
                    if l == 0:
                        xch = XS.tile([64, 512], F16, tag="xch")
                        nc.sync.dma_start(
                            xch[0:17, :], xTf_d.ap()[0:17, g * 512 : g * 512 + 512]
                        )
                        nc.sync.dma_start(
                            xch[32:49, :], xTf_d.ap()[17:34, g * 512 : g * 512 + 512]
                        )
                    for half in range(2):
                        stg = ST.tile([128, 4, 128], F16, tag="stg")
                        ps = MMP.tile([128, 512], FP32, tag="mm")
                        for q in range(4):
                            j = g * 4 + q
                            if l == 0:
                                lhsT = xch[half * 32 : half * 32 + 17,
                                           q * 128 : q * 128 + 128]
                            else:
                                lhsT = h16[half * 64 : half * 64 + 64,
                                           j * 128 : j * 128 + 128]
                            nc.tensor.matmul(
                                ps[:, q * 128 : q * 128 + w],
                                lhsT,
                                Wl_sb[half * KC : half * KC + KR, 0:w],
                                start=True, stop=True,
                            )
                        if w == 128:
                            use_dve = cfg.stg_mode == 1 or (
                                cfg.stg_mode == 2 and (g * 2 + half) % 2
                            )
                            if use_dve:
                                nc.vector.tensor_copy(
                                    stg.rearrange("p a b -> p (a b)"), ps[:, :]
                                )
                            else:
                                nc.scalar.copy(
                                    stg.rearrange("p a b -> p (a b)"), ps[:, :]
                                )
                        else:
                            for q in range(4):
                                nc.scalar.copy(
                                    stg[:, q : q + 1, 0:w].squeeze(1),
                                    ps[:, q * 128 : q * 128 + w],
                                )
                        slot0 = half * HALF2 + g * 512
                        dma_eng = (
                            nc.scalar if cfg.dma_spread and (g * 2 + half) % 2
                            else nc.sync
                        )
                        dma_eng.dma_start(
                            xl_tab[:]
                            .rearrange("(s p) c -> p s c", p=128)[
                                :, slot0 // 128 : slot0 // 128 + 4, 0:w
                            ],
                            stg[:, :, 0:w],
                        )

                # -------- per-block edge processing --------
                if cfg.stage < 2:
                    break
                for b in range(cfg.blocks):
                    K = int(cfg.K_sched[b])
                    # xr for this block: bias-seeded accumulating matmul
                    xr_ps = XRP.tile([128, 128], FP32, tag="xr")
                    nc.tensor.matmul(
                        xr_ps[:, 0:w], ones_sb[:], xrb_sb[:, 0:w],
                        start=True, stop=False,
                    )
                    loc = xloc_sb if l == 0 else h_loc
                    nc.tensor.matmul(
                        xr_ps[:, 0:w],
                        loc[0:KR, b * 128 : b * 128 + 128],
                        Wr_sb[0:KR, 0:w],
                        start=False, stop=True,
                    )
                    xr16 = SM.tile([128, 128], F16, tag="xr16")
                    nc.scalar.copy(xr16[:, 0:w], xr_ps[:, 0:w])
                    if cfg.bstage < 2:
                        continue

                    # gather xl[src] for the block's edge slots (fp16 rows)
                    if l not in cfg.gather_layers:
                        continue
                    gt = GP.tile([128, KMAX, 128], F16, tag="g")
                    nc.gpsimd.dma_gather(
                        gt[:, 0:K, :],
                        xl_tab[:, 0:128],
                        idx_sb[:, 8 * koff[b] : 8 * koff[b] + 8 * K],
                        128 * K, 128 * K, 128, elem_step=128, single_packet=False,
                    )

                    if cfg.bstage < 3:
                        continue
                    # s = lrelu(g + xr) * att   (all fp16)
                    s_t = SP.tile([128, KMAX, 128], F16, tag="s", name="s_t")
                    s = s_t[:, 0:K, 0:w]
                    nc.vector.tensor_tensor(
                        s, gt[:, 0:K, 0:w],
                        xr16[:, 0:w].unsqueeze(1).broadcast_to([128, K, w]),
                        ALU.add,
                    )
                    nc.scalar.activation(s, s, AF.Prelu, alpha=SLOPE)
                    satt_eng = nc.gpsimd if cfg.satt_on_pool else nc.vector
                    satt_eng.tensor_tensor(
                        s, s,
                        attR_sb[:, 0:w].unsqueeze(1).broadcast_to([128, K, w]),
                        ALU.mult,
                    )

                    if cfg.bstage < 4:
                        continue
                    # logit[d, k, h] (+ mask), fp32 accumulator
                    lg_t = SM.tile([128, KMAX, 2], FP32, tag="lg", name="lg_t")
                    lg = lg_t[:, 0:K, :]
                    lg_eng = nc.gpsimd if cfg.lg_on_pool else nc.vector
                    lg_eng.tensor_reduce(
                        lg, s.rearrange("p k (h c) -> p k h c", h=2), X, ALU.add,
                    )
                    nc.vector.tensor_tensor(
                        lg, lg,
                        mask_sb[:, koff[b] : koff[b] + K]
                        .unsqueeze(2).broadcast_to([128, K, 2]),
                        ALU.add,
                    )

                    if cfg.bstage < 5:
                        continue
                    # softmax over k per head; logits are clamped at 80 so exp
                    # cannot overflow fp32 (exact whenever logits stay < 80,
                    # graceful degradation instead of NaN beyond)
                    nc.vector.tensor_scalar_min(lg, lg, 80.0)
                    av_t = SM.tile([128, KMAX, 2], FP32, tag="av", name="av_t")
                    av = av_t[:, 0:K, :]
                    nc.scalar.activation(av, lg, AF.Exp)
                    ssum = SM.tile([128, 2], FP32, tag="ssum")
                    nc.vector.tensor_reduce(ssum[:], av.transpose([0, 2, 1]), X, ALU.add)
                    # head-mean 0.5 is folded into the o_slab write scale
                    rec = SM.tile([128, 2], FP32, tag="rec")
                    nc.vector.reciprocal(rec[:], ssum[:])
                    al16_t = SM.tile([128, KMAX, 2], F16, tag="al16", name="al16_t")
                    al16 = al16_t[:, 0:K, :]
                    nc.vector.tensor_tensor(
                        al16, av,
                        rec[:].unsqueeze(1).broadcast_to([128, K, 2]),
                        ALU.mult,
                    )

                    if cfg.bstage < 6:
                        continue
                    # weighted sum: g *= alpha (bcast over c, on gpsimd)
                    alf_b = al16.unsqueeze(3).broadcast_to([128, K, 2, w2])
                    g4 = gt[:, 0:K, 0:w].rearrange("p k (h c) -> p k h c", h=2)
                    use_dve = cfg.alpha_on_dve or K <= cfg.alpha_dve_kmax
                    eng = nc.vector if use_dve else nc.gpsimd
                    eng.tensor_tensor(g4, g4, alf_b, ALU.mult)
                    # single fp32 reduce over k (transposed view), then heads
                    ob_t = SM.tile([128, 64], FP32, tag="ob", name="ob_t")
                    ob = ob_t[:, 0:w2]
                    if cfg.red_xy:
                        nc.vector.tensor_reduce(
                            ob,
                            gt[:, 0:K, 0:w]
                            .rearrange("p k (h c) -> p k h c", h=2)
                            .transpose([0, 3, 1, 2]),
                            mybir.AxisListType.XY, ALU.add,
                        )
                    else:
                        red_t = SM.tile([128, 128], FP32, tag="red", name="red_t")
                        red = red_t[:, 0:w]
                        red_eng = nc.gpsimd if cfg.red_on_pool else nc.vector
                        red_eng.tensor_reduce(
                            red, gt[:, 0:K, 0:w].transpose([0, 2, 1]), X, ALU.add,
                        )
                        # head-mean (0.5 folded into the slab-write scale)
                        nc.vector.tensor_add(ob, red[:, 0:w2], red[:, w2:w])
                    if cfg.bstage < 7:
                        continue
                    # transpose to feature-major and add bias_eff (fp16 slab)
                    tp = TRP.tile([64, 128], FP32, tag="tp")
                    nc.tensor.transpose(tp[0:w2, :], ob, ident_sb[:])
                    nc.scalar.activation(
                        o_slab[0:w2, b * 128 : b * 128 + 128],
                        tp[0:w2, :], AF.Identity, scale=0.5,
                        bias=beff_sb[0:w2, :],
                    )

                # zero dead columns of the slab
                if cfg.real_per_core < SLOTS:
                    nc.vector.memset(o_slab[0:w2, cfg.real_per_core : SLOTS], 0.0)

                # -------- local BN partial sums (ride along the AllGather) ----
                if cfg.stage < 3:
                    break
                pp = SM.tile([64, 2], FP32, tag="pp")
                nc.vector.tensor_reduce(
                    pp[0:w2, 0:1], o_slab[0:w2, :], X, ALU.add,
                )
                scr = SP.tile([64, SLOTS], F16, tag="s")
                nc.scalar.activation(
                    scr[0:w2, :], o_slab[0:w2, :], AF.Square,
                    accum_out=pp[0:w2, 1:2],
                )

                # -------- AllGather of the fp16 pre-BN slab + partials --------
                agtag = "ag_in" if w2 == 64 else "ag_in_l"
                ag_in = DP.tile([1, w2 * SLOTS + w2 * 4], F16, tag=agtag,
                                bufs=2 if w2 == 64 else 1)
                nc.sync.dma_start(
                    ag_in[:, 0 : w2 * SLOTS].rearrange("o (p f) -> (o p) f", p=w2),
                    o_slab[0:w2, :],
                )
                nc.sync.dma_start(
                    ag_in[:, w2 * SLOTS :].rearrange("o (p f) -> (o p) f", p=w2),
                    pp[0:w2, :].bitcast(F16),
                )
                agotag = "ag_out" if w2 == 64 else "ag_out_l"
                ag_out = DP.tile([NC, w2 * SLOTS + w2 * 4], F16, tag=agotag,
                                 addr_space="Shared",
                                 bufs=2 if w2 == 64 else 1)
                nc.gpsimd.collective_compute(
                    "AllGather",
                    ALU.bypass,
                    ins=[ag_in.opt()],
                    outs=[ag_out.opt()],
                    replica_groups=[list(range(NC))],
                )

                # -------- land gathered slabs in SBUF (resident) --------
                nch = NC // 2
                ranges = [(0, 128)] if w2 == 64 else [(0, 32), (64, 96)]
                slabs = []
                for c4 in range(nch):
                    if cfg.slab_ring:
                        slc = OC.tile([128, SLOTS], F16, tag="slab_c",
                                      bufs=cfg.slab_ring)
                    else:
                        slc = slab16[:, c4 : c4 + 1, :].squeeze(1)
                    slabs.append(slc)
                    for hi in range(2):
                        dma_eng = (
                            nc.scalar if cfg.dma_spread and (c4 * 2 + hi) % 2
                            else nc.sync
                        )
                        dma_eng.dma_start(
                            slc[64 * hi : 64 * hi + w2, :],
                            ag_out[hi * nch + c4 : hi * nch + c4 + 1, 0 : w2 * SLOTS]
                            .rearrange("o (p f) -> (o p) f", p=w2),
                        )

                # -------- global BN stats from the gathered partials --------
                pt = SM.tile([64, 32], F16, tag="pt")
                nc.sync.dma_start(
                    pt[0:w2, :].rearrange("p (c f) -> p c f", c=NC),
                    ag_out[:, w2 * SLOTS :].rearrange("c (p f) -> p c f", p=w2),
                )
                s64 = SM.tile([64, 2], FP32, tag="s64")
                nc.vector.tensor_reduce(
                    s64[0:w2, :],
                    pt[0:w2, :].bitcast(FP32).rearrange("p (c j) -> p j c", j=2),
                    X, ALU.add,
                )

                # mu, var, scale, bias (on partitions 0:w2)
                stat = SM.tile([64, 4], FP32, tag="stat")
                nc.vector.tensor_scalar_mul(
                    stat[0:w2, 0:2], s64[0:w2, :], 1.0 / N_REAL
                )
                mu = stat[0:w2, 0:1]
                msq = stat[0:w2, 1:2]
                var = stat[0:w2, 2:3]
                nc.vector.tensor_tensor(var, mu, mu, ALU.mult)
                nc.vector.tensor_sub(var, msq, var)
                # rstd = exp(-0.5 * ln(var + eps))
                lnv = stat[0:w2, 3:4]
                nc.vector.tensor_scalar_add(var, var, float(EPS))
                nc.scalar.activation(lnv, var, AF.Ln)
                sc = SM.tile([128, 2], FP32, tag="sc")
                nc.scalar.activation(sc[0:w2, 0:1], lnv, AF.Exp, scale=-0.5)
                # scale = g * rstd ; bias = be - mu * scale
                nc.vector.tensor_tensor(
                    sc[0:w2, 0:1], sc[0:w2, 0:1], g_sb[0:w2, :], ALU.mult
                )
                nc.vector.tensor_tensor(sc[0:w2, 1:2], mu, sc[0:w2, 0:1], ALU.mult)
                nc.vector.tensor_sub(sc[0:w2, 1:2], be_sb[0:w2, :], sc[0:w2, 1:2])
                # replicate to fold partitions 64:64+w2
                nc.sync.dma_start(sc[64 : 64 + w2, :], sc[0:w2, :])

                # -------- h update (folded, all cores' columns) --------
                for c4 in range(nch):
                    sl = slabs[c4]
                    bn = OC.tile([128, SLOTS], F16, tag="bigs")
                    for (p0, p1) in ranges:
                        nc.scalar.activation(
                            bn[p0:p1, :], sl[p0:p1, :], AF.Relu,
                            scale=sc[p0:p1, 0:1], bias=sc[p0:p1, 1:2],
                        )
                        dst = h_fold[p0:p1, c4 * SLOTS : (c4 + 1) * SLOTS]
                        hadd_eng = nc.gpsimd if cfg.hadd_on_pool else nc.vector
                        if l == 0 or last:
                            hadd_eng.tensor_copy(dst, bn[p0:p1, :])
                        else:
                            hadd_eng.tensor_tensor(dst, dst, bn[p0:p1, :], ALU.add)
                        if not last:
                            h16dst = h16[p0:p1, c4 * SLOTS : (c4 + 1) * SLOTS]
                            if cfg.h16_on_pool:
                                nc.gpsimd.tensor_copy(h16dst, dst)
                            elif cfg.h16_on_dve:
                                nc.vector.tensor_copy(h16dst, dst)
                            else:
                                nc.scalar.copy(h16dst, dst)

                if dbg_d is not None and cfg.dump_layer == l:
                    nc.sync.dma_start(dbg_d.ap()[:, :], h_fold[:])

                # -------- h_loc update (from local fp16 slab) ----
                if not last:
                    bnl = OC.tile([128, SLOTS], F16, tag="bigs")
                    nc.scalar.activation(
                        bnl[0:64, :], o_slab[0:64, :], AF.Relu,
                        scale=sc[0:64, 0:1], bias=sc[0:64, 1:2],
                    )
                    if l == 0:
                        nc.vector.tensor_copy(h_loc[:], bnl[0:64, :])
                    else:
                        nc.vector.tensor_tensor(
                            h_loc[:], h_loc[:], bnl[0:64, :], ALU.add
                        )

            # ---------------- output heads ----------------
            headW_sb = P.tile([128, 2], FP32, tag="headW_sb")
            nc.sync.dma_start(headW_sb[:], headW_d[:, :])
            headb_sb = P.tile([2, 1], FP32, tag="headb_sb")
            nc.sync.dma_start(headb_sb[:], headb_d[:, :])
            for half in range(2):
                base = 64 * half
                for j in range(HALF2 // 512):
                    hp = TRP.tile([2, 512], FP32, tag="tp")
                    nc.tensor.matmul(
                        hp[:],
                        headW_sb[base : base + 32, :],
                        h_fold[base : base + 32, j * 512 : (j + 1) * 512],
                        start=True, stop=True,
                    )
                    hs = SM.tile([2, 512], FP32, tag="hs")
                    nc.scalar.activation(hs[:], hp[:], AF.Identity, bias=headb_sb[:])
                    hs2 = SM.tile([2, 512], FP32, tag="hs2")
                    nc.scalar.activation(hs2[:], hs[:], AF.Sigmoid)
                    # row 0 = imp (linear), row 1 = pol (sigmoid)
                    col0 = half * HALF2 + j * 512
                    nc.sync.dma_start(out_d.ap()[0:1, col0 : col0 + 512], hs[0:1, :])
                    nc.sync.dma_start(out_d.ap()[1:2, col0 : col0 + 512], hs2[1:2, :])

    nc.compile()
    return nc


# ===================== host side =====================

def make_cfg(deg, ncores=8, nlayers=20, f_in=17):
    n = deg.shape[0]
    real = n // ncores
    blocks = (real + 127) // 128
    order = np.argsort(deg, kind="stable")
    Ks = np.zeros((ncores, blocks), np.int64)
    for c in range(ncores):
        dc = deg[order[c::ncores]]
        for b in range(blocks):
            blk = dc[b * 128 : (b + 1) * 128]
            Ks[c, b] = blk.max() if blk.size else 1
    K_sched = tuple(int(max(x, 1)) for x in Ks.max(axis=0))
    cfg = Cfg(ncores=ncores, blocks=blocks, real_per_core=real,
              nlayers=nlayers, K_sched=K_sched, f_in=f_in)
    return order, cfg


def host_prep(inputs, nlayers=20, ncores=8):
    """Build cfg, per-core input maps, and the slot->node mapping."""
    x = np.asarray(inputs["x"], np.float32)
    src = np.asarray(inputs["src"], np.int64)
    dst = np.asarray(inputs["dst"], np.int64)
    n = x.shape[0]
    loop = np.arange(n, dtype=np.int64)
    s_all = np.concatenate([src, loop])
    d_all = np.concatenate([dst, loop])
    deg = np.bincount(d_all, minlength=n)

    order, cfg = make_cfg(deg, ncores=ncores, nlayers=nlayers, f_in=x.shape[1])
    SLOTS = cfg.slots
    NPAD = cfg.npad
    L = nlayers

    slot_of_node = np.full(n, -1, np.int64)
    for c in range(ncores):
        nodes = order[c::ncores]
        slot_of_node[nodes] = c * SLOTS + np.arange(nodes.shape[0])
    assert (slot_of_node >= 0).all()

    s_slot = slot_of_node[s_all]
    d_slot = slot_of_node[d_all]

    Ksch = cfg.K_sched
    sumk = cfg.sumk
    koff = np.concatenate([[0], np.cumsum(Ksch)]).astype(np.int64)
    order_e = np.argsort(d_slot, kind="stable")
    ds_sorted = d_slot[order_e]
    ss_sorted = s_slot[order_e]
    starts = np.searchsorted(ds_sorted, np.arange(NPAD))
    ends = np.searchsorted(ds_sorted, np.arange(NPAD) + 1)

    idx_maps, mask_maps = [], []
    for c in range(ncores):
        mask = np.full((128, sumk), np.float32(NEG16), np.float32)
        idx_arr = np.zeros((16, 8 * sumk), np.int16)
        for b in range(cfg.blocks):
            K = int(Ksch[b])
            flat = np.zeros(128 * K, np.int16)
            for lane in range(128):
                sl = c * SLOTS + b * 128 + lane
                e0, e1 = starts[sl], ends[sl]
                kdeg = e1 - e0
                assert kdeg <= K, (kdeg, K, b)
                if kdeg:
                    flat[np.arange(kdeg) * 128 + lane] = ss_sorted[e0:e1].astype(np.int16)
                    mask[lane, koff[b] : koff[b] + kdeg] = 0.0
            blk = flat.reshape(8 * K, 16)
            idx_arr[:, 8 * koff[b] : 8 * koff[b] + 8 * K] = blk.T
        idx_maps.append(idx_arr)
        mask_maps.append(mask.astype(F16NP))

    xp = np.zeros((NPAD, 32), np.float32)
    xp[slot_of_node[np.arange(n)], : x.shape[1]] = x
    HALF2 = cfg.half2
    f_in = x.shape[1]
    xTf = np.ascontiguousarray(
        np.concatenate([xp[:HALF2, :f_in].T, xp[HALF2:, :f_in].T], axis=0)
    ).astype(F16NP)
    xloc_maps = [
        np.ascontiguousarray(xp[c * SLOTS : (c + 1) * SLOTS, :f_in].T)
        for c in range(ncores)
    ]

    def wpack(W, KC):
        out = np.zeros((64, 128), np.float32)
        kin, wout = W.shape
        out[0:kin, 0:wout] = W
        return out

    Wl_all = np.zeros((L, 64, 128), np.float32)
    Wr_all = np.zeros((L, 64, 128), np.float32)
    attR = np.zeros((L, 1, 128), np.float32)
    xrb = np.zeros((L, 128), np.float32)
    beff = np.zeros((L, 64), np.float32)
    g_all = np.zeros((L, 64), np.float32)
    be_all = np.zeros((L, 64), np.float32)

    def layer_params(l):
        if l == 0:
            return (inputs["W0l"], inputs["b0l"], inputs["W0r"], inputs["b0r"],
                    inputs["att0"], inputs["bias0"], inputs["g0"], inputs["be0"])
        if l < L - 1:
            i = l - 1
            return (inputs["Wml"][i], inputs["bml"][i], inputs["Wmr"][i],
                    inputs["bmr"][i], inputs["attm"][i], inputs["biasm"][i],
                    inputs["gm"][i], inputs["bem"][i])
        return (inputs["WLl"], inputs["bLl"], inputs["WLr"], inputs["bLr"],
                inputs["attL"], inputs["biasL"], inputs["gL"], inputs["beL"])

    for l in range(L):
        Wl, bl, Wr, br, att, bias, g, be = [
            np.asarray(a, np.float32) for a in layer_params(l)
        ]
        KC = 32 if l == 0 else 64
        w = 128 if l < L - 1 else 64
        w2 = w // 2
        Wl_all[l] = wpack(Wl, KC)
        Wr_all[l] = wpack(Wr, KC)
        attR[l, 0, 0:w] = att.reshape(w)
        xrb[l, 0:w] = bl + br
        beff[l, 0:w2] = bias + 0.5 * (bl.reshape(2, w2)[0] + bl.reshape(2, w2)[1])
        g_all[l, 0:w2] = g
        be_all[l, 0:w2] = be

    headW = np.zeros((128, 2), np.float32)
    Wimp = np.asarray(inputs["Wimp"], np.float32)
    Wpol = np.asarray(inputs["Wpol"], np.float32)
    d_out = Wimp.shape[0]
    headW[0:d_out, 0] = Wimp[:, 0]
    headW[0:d_out, 1] = Wpol[:, 0]
    headW[64 : 64 + d_out, 0] = Wimp[:, 0]
    headW[64 : 64 + d_out, 1] = Wpol[:, 0]
    headb = np.array(
        [[np.float32(np.asarray(inputs["bimp"]).reshape(-1)[0])],
         [np.float32(np.asarray(inputs["bpol"]).reshape(-1)[0])]], np.float32
    )

    shared = dict(
        xTf=xTf,
        Wl=Wl_all.astype(F16NP), Wr=Wr_all, attR=attR,
        xrb=xrb, beff=beff,
        gbn=g_all, bebn=be_all,
        ident=np.eye(128, dtype=np.float32),
        headW=headW, headb=headb,
    )
    in_maps = []
    for c in range(ncores):
        m = dict(shared)
        m["idx"] = idx_maps[c]
        m["mask"] = mask_maps[c]
        m["xloc"] = xloc_maps[c]
        in_maps.append(m)
    return cfg, in_maps, slot_of_node


# ===================== entry point =====================

LAST_EXEC_NS = None
_TRACE = bool(int(__import__("os").environ.get("GNN_TRACE", "0")))
_NC_CACHE = {}


def _built(cfg):
    # memoize by the cfg fields that determine the generated program, so
    # repeated kernel() calls in one process skip the ~2s rebuild
    key = (cfg.ncores, cfg.blocks, cfg.real_per_core, cfg.nlayers,
           cfg.K_sched, cfg.f_in)
    nc = _NC_CACHE.get(key)
    if nc is None:
        nc = _NC_CACHE[key] = build_kernel(cfg)
    return nc


def kernel(**inputs):
    """Full-input GATv2 GNN on 8 trn2 NeuronCores; returns (imp, pol)."""
    global LAST_EXEC_NS
    from concourse.bass_utils import run_bass_kernel_spmd

    cfg, in_maps, slot_of_node = host_prep(inputs, nlayers=20, ncores=8)
    nc = _built(cfg)
    res = run_bass_kernel_spmd(
        nc, in_maps, core_ids=list(range(cfg.ncores)), trace=_TRACE
    )
    LAST_EXEC_NS = res.exec_time_ns
    out = np.asarray(res.results[0]["out"])
    imp = out[0][slot_of_node][:, None]
    pol = out[1][slot_of_node][:, None]
    return imp.astype(np.float32), pol.astype(np.float32)



# revision 39
# speedup vs baseline: 1.1116x; 1.1116x over previous
"""Distributed GATv2 GNN kernel for trn2 (8 NeuronCores).

Sharding: nodes are degree-sorted and striped across 8 cores (graph
partition by destination). Each core processes the incoming-edge segments
of its nodes in blocks of 128 dst lanes with a uniform per-block padded
degree schedule K_sched. Per layer:
  - every core computes the full xl = h @ Wl table (fp16, node-major,
    DRAM) with fp16 matmuls,
  - per dst block: dma_gather of fp16 xl[src] rows, GATv2 attention +
    softmax in fp16/fp32 on DVE/ACT, alpha-weighted sum accumulated by a
    single fp32 TensorReduce over the degree axis, PE transpose into a
    feature-major fp16 local slab,
  - AllGather of the fp16 pre-BN slab; gathered slabs stay resident in
    SBUF: global BN stats + ReLU (+ residual into fp32 h) computed
    redundantly on every core, with a fp16 shadow h16 feeding the next
    layer's matmuls.
Weights are replicated. Host-staged tensors are deduplicated (idx 16-row
stream, att rows, single-copy Wl/Wr halves, 17-feature x slices) and
re-expanded on device, cutting host->device staging from 41.5 MB to
17.1 MB across the 8 cores. imp/pol are packed into one [2, NPAD] output
tensor so the result fetch is a single sharded-array pull.
"""
import numpy as np
from dataclasses import dataclass

import concourse.bass as bass
import concourse.bacc as bacc
import concourse.tile as tile
import concourse.mybir as mybir

AF = mybir.ActivationFunctionType
ALU = mybir.AluOpType
FP32 = mybir.dt.float32
F16 = mybir.dt.float16
I16 = mybir.dt.int16

SLOPE = 0.2
EPS = 1e-5
NEG = -1.0e30
NEG16 = -60000.0
F16NP = np.float16


@dataclass
class Cfg:
    ncores: int = 8
    blocks: int = 20               # dst blocks per core
    real_per_core: int = 2500      # real nodes per core
    nlayers: int = 20              # total GAT layers (first + mids + final)
    K_sched: tuple = ()            # per-block padded degree (shared by cores)
    f_in: int = 17
    stage: int = 4
    bstage: int = 7
    gather_layers: tuple = tuple(range(32))
    dump_layer: int = -1
    gp_bufs: int = 4
    sp_bufs: int = 2
    sm_bufs: int = 4
    mmp_bufs: int = 4
    trp_bufs: int = 2
    oc_bufs: int = 2
    st_bufs: int = 6
    alpha_on_dve: bool = True
    alpha_dve_kmax: int = 0        # blocks with K <= this use DVE for alpha
    dma_spread: bool = False       # alternate bulk DMAs across SP/ACT queues
    stg_mode: int = 0              # 0=ACT, 1=DVE, 2=alternate ACT/DVE
    h16_on_pool: bool = True       # h16 fp16 shadow copies on gpsimd
    h16_on_dve: bool = False       # h16 fp16 shadow copies on DVE
    red_on_pool: bool = False      # weighted-sum reduce on gpsimd
    lg_on_pool: bool = False       # logit reduce on gpsimd
    satt_on_pool: bool = False     # s*att multiply on gpsimd
    hadd_on_pool: bool = False     # h_fold residual adds on gpsimd
    red_xy: bool = False           # fuse weighted-sum reduce + head sum (XY)
    xl_dma_gpsimd: bool = False    # alternate xl-tab writes SP/gpsimd queues
    slab_ring: int = 2             # 0 = persistent slab16, else ring bufs

    @property
    def slots(self):
        return self.blocks * 128

    @property
    def npad(self):
        return self.ncores * self.slots

    @property
    def half2(self):
        return self.npad // 2

    @property
    def sumk(self):
        return int(sum(self.K_sched))

    def width(self, l):           # H*D of layer l
        return 128 if l < self.nlayers - 1 else 64

    def kcontract(self, l):       # matmul contraction dim
        return 32 if l == 0 else 64


def build_kernel(cfg: Cfg):
    NC = cfg.ncores
    SLOTS = cfg.slots
    NPAD = cfg.npad
    HALF2 = cfg.half2
    L = cfg.nlayers
    SUMK = cfg.sumk
    KMAX = int(max(cfg.K_sched))
    N_REAL = NC * cfg.real_per_core
    X = mybir.AxisListType.X

    nc = bacc.Bacc("TRN2", target_bir_lowering=False, debug=False, num_devices=NC)

    # ---------------- DRAM I/O ----------------
    # host stages deduplicated tensors; the kernel re-expands on device
    idx_d = nc.dram_tensor("idx", [16, 8 * SUMK], I16, kind="ExternalInput")
    mask_d = nc.dram_tensor("mask", [128, SUMK], F16, kind="ExternalInput")
    xTf_d = nc.dram_tensor("xTf", [34, HALF2], F16, kind="ExternalInput")
    xloc_d = nc.dram_tensor("xloc", [17, SLOTS], FP32, kind="ExternalInput")
    Wl_d = nc.dram_tensor("Wl", [L, 64, 128], F16, kind="ExternalInput")
    Wr_d = nc.dram_tensor("Wr", [L, 64, 128], FP32, kind="ExternalInput")
    attR_d = nc.dram_tensor("attR", [L, 1, 128], FP32, kind="ExternalInput")
    xrb_d = nc.dram_tensor("xrb", [L, 128], FP32, kind="ExternalInput")
    beff_d = nc.dram_tensor("beff", [L, 64], FP32, kind="ExternalInput")
    g_d = nc.dram_tensor("gbn", [L, 64], FP32, kind="ExternalInput")
    be_d = nc.dram_tensor("bebn", [L, 64], FP32, kind="ExternalInput")
    ident_d = nc.dram_tensor("ident", [128, 128], FP32, kind="ExternalInput")
    headW_d = nc.dram_tensor("headW", [128, 2], FP32, kind="ExternalInput")
    headb_d = nc.dram_tensor("headb", [2, 1], FP32, kind="ExternalInput")

    out_d = nc.dram_tensor("out", [2, NPAD], FP32, kind="ExternalOutput")
    dbg_d = None
    if cfg.dump_layer >= 0:
        dbg_d = nc.dram_tensor("out_dbg", [128, HALF2], FP32, kind="ExternalOutput")

    with tile.TileContext(nc) as tc:
        with (
            tc.tile_pool(name="persist", bufs=1) as P,
            tc.tile_pool(name="wload", bufs=2) as WP,
            tc.tile_pool(name="gpool", bufs=cfg.gp_bufs) as GP,
            tc.tile_pool(name="spool", bufs=cfg.sp_bufs) as SP,
            tc.tile_pool(name="small", bufs=cfg.sm_bufs) as SM,
            tc.tile_pool(name="stage", bufs=cfg.st_bufs) as ST,
            tc.tile_pool(name="ochunk", bufs=cfg.oc_bufs) as OC,
            tc.tile_pool(name="xstream", bufs=2) as XS,
            tc.tile_pool(name="mm_ps", bufs=cfg.mmp_bufs, space="PSUM") as MMP,
            tc.tile_pool(name="xr_ps", bufs=2, space="PSUM") as XRP,
            tc.tile_pool(name="tr_ps", bufs=cfg.trp_bufs, space="PSUM") as TRP,
            tc.tile_pool(name="dram", bufs=2, space="DRAM") as DP,
        ):
            # ---------------- persistent SBUF ----------------
            h_fold = P.tile([128, HALF2], FP32, tag="h_fold")
            h16 = P.tile([128, HALF2], F16, tag="h16")
            h_loc = P.tile([64, SLOTS], FP32, tag="h_loc")
            idx_sb = P.tile([128, 8 * SUMK], I16, tag="idx_sb")
            mask_sb = P.tile([128, SUMK], F16, tag="mask_sb")
            ones_sb = P.tile([1, 128], FP32, tag="ones_sb")
            ident_sb = P.tile([128, 128], FP32, tag="ident_sb")
            o_slab = P.tile([64, SLOTS], F16, tag="o_slab")
            slab16 = None
            if not cfg.slab_ring:
                slab16 = P.tile([128, 4, SLOTS], F16, tag="slab16")
            xloc_sb = P.tile([17, SLOTS], FP32, tag="xloc_sb")

            # expand host-deduplicated inputs on device
            for i in range(8):
                nc.sync.dma_start(idx_sb[16 * i : 16 * i + 16, :], idx_d[:, :])
            nc.sync.dma_start(mask_sb[:], mask_d[:, :])
            nc.sync.dma_start(ident_sb[:], ident_d[:, :])
            nc.sync.dma_start(xloc_sb[:], xloc_d[:, :])
            nc.vector.memset(ones_sb[:], 1.0)
            if cfg.stage < 4:
                nc.vector.memset(h_fold[:], 0.0)
                nc.vector.memset(h16[:], 0.0)
                nc.vector.memset(h_loc[:], 0.0)
                nc.vector.memset(o_slab[:], 0.0)

            koff = [0]
            for K in cfg.K_sched:
                koff.append(koff[-1] + int(K))

            for l in range(L):
                w = cfg.width(l)       # H*D of this layer
                w2 = w // 2            # per-head width = output width
                KC = cfg.kcontract(l)  # matmul contraction
                last = l == L - 1

                # -------- per-layer weight loads (duplicate halves on device) --
                Wl_sb = WP.tile([128, 128], F16, tag="Wl_sb")
                nc.sync.dma_start(
                    Wl_sb[0:KC, :], Wl_d.ap()[l : l + 1, 0:KC].squeeze(0)
                )
                nc.sync.dma_start(
                    Wl_sb[KC : 2 * KC, :], Wl_d.ap()[l : l + 1, 0:KC].squeeze(0)
                )
                Wr_sb = WP.tile([128, 128], FP32, tag="Wr_sb")
                nc.sync.dma_start(
                    Wr_sb[0:KC, :], Wr_d.ap()[l : l + 1, 0:KC].squeeze(0)
                )
                nc.sync.dma_start(
                    Wr_sb[KC : 2 * KC, :], Wr_d.ap()[l : l + 1, 0:KC].squeeze(0)
                )
                # broadcast att row to all 128 partitions via rank-1 matmul
                attRow = WP.tile([1, 128], FP32, tag="attRow")
                nc.sync.dma_start(
                    attRow[:], attR_d.ap()[l : l + 1, :, :].squeeze(0)
                )
                attb_ps = XRP.tile([128, 128], FP32, tag="xr")
                nc.tensor.matmul(
                    attb_ps[:, 0:w], ones_sb[:], attRow[:, 0:w],
                    start=True, stop=True,
                )
                attR_sb = WP.tile([128, 128], F16, tag="attR_sb")
                nc.scalar.copy(attR_sb[:, 0:w], attb_ps[:, 0:w])
                xrb_sb = WP.tile([1, 128], FP32, tag="xrb_sb")
                nc.sync.dma_start(xrb_sb[:], xrb_d.ap()[l : l + 1, :])
                beff_sb = WP.tile([64, 1], FP32, tag="beff_sb")
                nc.sync.dma_start(beff_sb[:], beff_d.ap()[l : l + 1, :].rearrange("o f -> f o"))
                g_sb = WP.tile([64, 1], FP32, tag="g_sb")
                nc.sync.dma_start(g_sb[:], g_d.ap()[l : l + 1, :].rearrange("o f -> f o"))
                be_sb = WP.tile([64, 1], FP32, tag="be_sb")
                nc.sync.dma_start(be_sb[:], be_d.ap()[l : l + 1, :].rearrange("o f -> f o"))

                # -------- xl table: [NPAD, 128-pitch] fp16 in DRAM --------
                xl_tab = DP.tile([NPAD, 128], F16, tag="xl_tab")
                n_groups = HALF2 // 512
                # layer 0 contracts over the 17 real input features only
                KR = 17 if l == 0 else KC
                for g in range(n_groups):
                    if l == 0:
                        xch = XS.tile([64, 512], F16, tag="xch")
                        nc.sync.dma_start(
                            xch[0:17, :], xTf_d.ap()[0:17, g * 512 : g * 512 + 512]
                        )
                        nc.sync.dma_start(
                            xch[32:49, :], xTf_d.ap()[17:34, g * 512 : g * 512 + 512]
                        )
                    for half in range(2):
                        stg = ST.tile([128, 4, 128], F16, tag="stg")
                        ps = MMP.tile([128, 512], FP32, tag="mm")
                        for q in range(4):
                            j = g * 4 + q
                            if l == 0:
                                lhsT = xch[half * 32 : half * 32 + 17,
                                           q * 128 : q * 128 + 128]
                            else:
                                lhsT = h16[half * 64 : half * 64 + 64,
                                           j * 128 : j * 128 + 128]
                            nc.tensor.matmul(
                                ps[:, q * 128 : q * 128 + w],
                                lhsT,
                                Wl_sb[half * KC : half * KC + KR, 0:w],
                                start=True, stop=True,
                            )
                        if w == 128:
                            use_dve = cfg.stg_mode == 1 or (
                                cfg.stg_mode == 2 and (g * 2 + half) % 2
                            )
                            if use_dve:
                                nc.vector.tensor_copy(
                                    stg.rearrange("p a b -> p (a b)"), ps[:, :]
                                )
                            else:
                                nc.scalar.copy(
                                    stg.rearrange("p a b -> p (a b)"), ps[:, :]
                                )
                        else:
                            for q in range(4):
                                nc.scalar.copy(
                                    stg[:, q : q + 1, 0:w].squeeze(1),
                                    ps[:, q * 128 : q * 128 + w],
                                )
                        slot0 = half * HALF2 + g * 512
                        if cfg.xl_dma_gpsimd and (g * 2 + half) % 2:
                            dma_eng = nc.gpsimd
                        elif cfg.dma_spread and (g * 2 + half) % 2:
                            dma_eng = nc.scalar
                        else:
                            dma_eng = nc.sync
                        dma_eng.dma_start(
                            xl_tab[:]
                            .rearrange("(s p) c -> p s c", p=128)[
                                :, slot0 // 128 : slot0 // 128 + 4, 0:w
                            ],
                            stg[:, :, 0:w],
                        )

                # -------- per-block edge processing --------
                if cfg.stage < 2:
                    break
                for b in range(cfg.blocks):
                    K = int(cfg.K_sched[b])
                    # xr for this block: bias-seeded accumulating matmul
                    xr_ps = XRP.tile([128, 128], FP32, tag="xr")
                    nc.tensor.matmul(
                        xr_ps[:, 0:w], ones_sb[:], xrb_sb[:, 0:w],
                        start=True, stop=False,
                    )
                    loc = xloc_sb if l == 0 else h_loc
                    nc.tensor.matmul(
                        xr_ps[:, 0:w],
                        loc[0:KR, b * 128 : b * 128 + 128],
                        Wr_sb[0:KR, 0:w],
                        start=False, stop=True,
                    )
                    xr16 = SM.tile([128, 128], F16, tag="xr16")
                    nc.scalar.copy(xr16[:, 0:w], xr_ps[:, 0:w])
                    if cfg.bstage < 2:
                        continue

                    # gather xl[src] for the block's edge slots (fp16 rows)
                    if l not in cfg.gather_layers:
                        continue
                    gt = GP.tile([128, KMAX, 128], F16, tag="g")
                    nc.gpsimd.dma_gather(
                        gt[:, 0:K, :],
                        xl_tab[:, 0:128],
                        idx_sb[:, 8 * koff[b] : 8 * koff[b] + 8 * K],
                        128 * K, 128 * K, 128, elem_step=128, single_packet=False,
                    )

                    if cfg.bstage < 3:
                        continue
                    # s = lrelu(g + xr) * att   (all fp16)
                    s_t = SP.tile([128, KMAX, 128], F16, tag="s", name="s_t")
                    s = s_t[:, 0:K, 0:w]
                    nc.vector.tensor_tensor(
                        s, gt[:, 0:K, 0:w],
                        xr16[:, 0:w].unsqueeze(1).broadcast_to([128, K, w]),
                        ALU.add,
                    )
                    nc.scalar.activation(s, s, AF.Prelu, alpha=SLOPE)
                    satt_eng = nc.gpsimd if cfg.satt_on_pool else nc.vector
                    satt_eng.tensor_tensor(
                        s, s,
                        attR_sb[:, 0:w].unsqueeze(1).broadcast_to([128, K, w]),
                        ALU.mult,
                    )

                    if cfg.bstage < 4:
                        continue
                    # logit[d, k, h] (+ mask), fp32 accumulator
                    lg_t = SM.tile([128, KMAX, 2], FP32, tag="lg", name="lg_t")
                    lg = lg_t[:, 0:K, :]
                    lg_eng = nc.gpsimd if cfg.lg_on_pool else nc.vector
                    lg_eng.tensor_reduce(
                        lg, s.rearrange("p k (h c) -> p k h c", h=2), X, ALU.add,
                    )
                    nc.vector.tensor_tensor(
                        lg, lg,
                        mask_sb[:, koff[b] : koff[b] + K]
                        .unsqueeze(2).broadcast_to([128, K, 2]),
                        ALU.add,
                    )

                    if cfg.bstage < 5:
                        continue
                    # softmax over k per head; logits are clamped at 80 so exp
                    # cannot overflow fp32 (exact whenever logits stay < 80,
                    # graceful degradation instead of NaN beyond)
                    nc.vector.tensor_scalar_min(lg, lg, 80.0)
                    av_t = SM.tile([128, KMAX, 2], FP32, tag="av", name="av_t")
                    av = av_t[:, 0:K, :]
                    nc.scalar.activation(av, lg, AF.Exp)
                    ssum = SM.tile([128, 2], FP32, tag="ssum")
                    nc.vector.tensor_reduce(ssum[:], av.transpose([0, 2, 1]), X, ALU.add)
                    # head-mean 0.5 is folded into the o_slab write scale
                    rec = SM.tile([128, 2], FP32, tag="rec")
                    nc.vector.reciprocal(rec[:], ssum[:])
                    al16_t = SM.tile([128, KMAX, 2], F16, tag="al16", name="al16_t")
                    al16 = al16_t[:, 0:K, :]
                    nc.vector.tensor_tensor(
                        al16, av,
                        rec[:].unsqueeze(1).broadcast_to([128, K, 2]),
                        ALU.mult,
                    )

                    if cfg.bstage < 6:
                        continue
                    # weighted sum: g *= alpha (bcast over c, on gpsimd)
                    alf_b = al16.unsqueeze(3).broadcast_to([128, K, 2, w2])
                    g4 = gt[:, 0:K, 0:w].rearrange("p k (h c) -> p k h c", h=2)
                    use_dve = cfg.alpha_on_dve or K <= cfg.alpha_dve_kmax
                    eng = nc.vector if use_dve else nc.gpsimd
                    eng.tensor_tensor(g4, g4, alf_b, ALU.mult)
                    # single fp32 reduce over k (transposed view), then heads
                    ob_t = SM.tile([128, 64], FP32, tag="ob", name="ob_t")
                    ob = ob_t[:, 0:w2]
                    if cfg.red_xy:
                        nc.vector.tensor_reduce(
                            ob,
                            gt[:, 0:K, 0:w]
                            .rearrange("p k (h c) -> p k h c", h=2)
                            .transpose([0, 3, 1, 2]),
                            mybir.AxisListType.XY, ALU.add,
                        )
                    else:
                        red_t = SM.tile([128, 128], FP32, tag="red", name="red_t")
                        red = red_t[:, 0:w]
                        red_eng = nc.gpsimd if cfg.red_on_pool else nc.vector
                        red_eng.tensor_reduce(
                            red, gt[:, 0:K, 0:w].transpose([0, 2, 1]), X, ALU.add,
                        )
                        # head-mean (0.5 folded into the slab-write scale)
                        nc.vector.tensor_add(ob, red[:, 0:w2], red[:, w2:w])
                    if cfg.bstage < 7:
                        continue
                    # transpose to feature-major and add bias_eff (fp16 slab)
                    tp = TRP.tile([64, 128], FP32, tag="tp")
                    nc.tensor.transpose(tp[0:w2, :], ob, ident_sb[:])
                    nc.scalar.activation(
                        o_slab[0:w2, b * 128 : b * 128 + 128],
                        tp[0:w2, :], AF.Identity, scale=0.5,
                        bias=beff_sb[0:w2, :],
                    )

                # zero dead columns of the slab
                if cfg.real_per_core < SLOTS:
                    nc.vector.memset(o_slab[0:w2, cfg.real_per_core : SLOTS], 0.0)

                # -------- local BN partial sums (ride along the AllGather) ----
                if cfg.stage < 3:
                    break
                pp = SM.tile([64, 2], FP32, tag="pp")
                nc.vector.tensor_reduce(
                    pp[0:w2, 0:1], o_slab[0:w2, :], X, ALU.add,
                )
                scr = SP.tile([64, SLOTS], F16, tag="s")
                nc.scalar.activation(
                    scr[0:w2, :], o_slab[0:w2, :], AF.Square,
                    accum_out=pp[0:w2, 1:2],
                )

                # -------- AllGather of the fp16 pre-BN slab + partials --------
                agtag = "ag_in" if w2 == 64 else "ag_in_l"
                ag_in = DP.tile([1, w2 * SLOTS + w2 * 4], F16, tag=agtag,
                                bufs=2 if w2 == 64 else 1)
                nc.sync.dma_start(
                    ag_in[:, 0 : w2 * SLOTS].rearrange("o (p f) -> (o p) f", p=w2),
                    o_slab[0:w2, :],
                )
                nc.sync.dma_start(
                    ag_in[:, w2 * SLOTS :].rearrange("o (p f) -> (o p) f", p=w2),
                    pp[0:w2, :].bitcast(F16),
                )
                agotag = "ag_out" if w2 == 64 else "ag_out_l"
                ag_out = DP.tile([NC, w2 * SLOTS + w2 * 4], F16, tag=agotag,
                                 addr_space="Shared",
                                 bufs=2 if w2 == 64 else 1)
                nc.gpsimd.collective_compute(
                    "AllGather",
                    ALU.bypass,
                    ins=[ag_in.opt()],
                    outs=[ag_out.opt()],
                    replica_groups=[list(range(NC))],
                )

                # -------- land gathered slabs in SBUF (resident) --------
                nch = NC // 2
                ranges = [(0, 128)] if w2 == 64 else [(0, 32), (64, 96)]
                slabs = []
                for c4 in range(nch):
                    if cfg.slab_ring:
                        slc = OC.tile([128, SLOTS], F16, tag="slab_c",
                                      bufs=cfg.slab_ring)
                    else:
                        slc = slab16[:, c4 : c4 + 1, :].squeeze(1)
                    slabs.append(slc)
                    for hi in range(2):
                        dma_eng = (
                            nc.scalar if cfg.dma_spread and (c4 * 2 + hi) % 2
                            else nc.sync
                        )
                        dma_eng.dma_start(
                            slc[64 * hi : 64 * hi + w2, :],
                            ag_out[hi * nch + c4 : hi * nch + c4 + 1, 0 : w2 * SLOTS]
                            .rearrange("o (p f) -> (o p) f", p=w2),
                        )

                # -------- global BN stats from the gathered partials --------
                pt = SM.tile([64, 32], F16, tag="pt")
                nc.sync.dma_start(
                    pt[0:w2, :].rearrange("p (c f) -> p c f", c=NC),
                    ag_out[:, w2 * SLOTS :].rearrange("c (p f) -> p c f", p=w2),
                )
                s64 = SM.tile([64, 2], FP32, tag="s64")
                nc.vector.tensor_reduce(
                    s64[0:w2, :],
                    pt[0:w2, :].bitcast(FP32).rearrange("p (c j) -> p j c", j=2),
                    X, ALU.add,
                )

                # mu, var, scale, bias (on partitions 0:w2)
                stat = SM.tile([64, 4], FP32, tag="stat")
                nc.vector.tensor_scalar_mul(
                    stat[0:w2, 0:2], s64[0:w2, :], 1.0 / N_REAL
                )
                mu = stat[0:w2, 0:1]
                msq = stat[0:w2, 1:2]
                var = stat[0:w2, 2:3]
                nc.vector.tensor_tensor(var, mu, mu, ALU.mult)
                nc.vector.tensor_sub(var, msq, var)
                # rstd = exp(-0.5 * ln(var + eps))
                lnv = stat[0:w2, 3:4]
                nc.vector.tensor_scalar_add(var, var, float(EPS))
                nc.scalar.activation(lnv, var, AF.Ln)
                sc = SM.tile([128, 2], FP32, tag="sc")
                nc.scalar.activation(sc[0:w2, 0:1], lnv, AF.Exp, scale=-0.5)
                # scale = g * rstd ; bias = be - mu * scale
                nc.vector.tensor_tensor(
                    sc[0:w2, 0:1], sc[0:w2, 0:1], g_sb[0:w2, :], ALU.mult
                )
                nc.vector.tensor_tensor(sc[0:w2, 1:2], mu, sc[0:w2, 0:1], ALU.mult)
                nc.vector.tensor_sub(sc[0:w2, 1:2], be_sb[0:w2, :], sc[0:w2, 1:2])
                # replicate to fold partitions 64:64+w2
                nc.sync.dma_start(sc[64 : 64 + w2, :], sc[0:w2, :])

                # -------- h update (folded, all cores' columns) --------
                for c4 in range(nch):
                    sl = slabs[c4]
                    bn = OC.tile([128, SLOTS], F16, tag="bigs")
                    for (p0, p1) in ranges:
                        nc.scalar.activation(
                            bn[p0:p1, :], sl[p0:p1, :], AF.Relu,
                            scale=sc[p0:p1, 0:1], bias=sc[p0:p1, 1:2],
                        )
                        dst = h_fold[p0:p1, c4 * SLOTS : (c4 + 1) * SLOTS]
                        hadd_eng = nc.gpsimd if cfg.hadd_on_pool else nc.vector
                        if l == 0 or last:
                            hadd_eng.tensor_copy(dst, bn[p0:p1, :])
                        else:
                            hadd_eng.tensor_tensor(dst, dst, bn[p0:p1, :], ALU.add)
                        if not last:
                            h16dst = h16[p0:p1, c4 * SLOTS : (c4 + 1) * SLOTS]
                            if cfg.h16_on_pool:
                                nc.gpsimd.tensor_copy(h16dst, dst)
                            elif cfg.h16_on_dve:
                                nc.vector.tensor_copy(h16dst, dst)
                            else:
                                nc.scalar.copy(h16dst, dst)

                if dbg_d is not None and cfg.dump_layer == l:
                    nc.sync.dma_start(dbg_d.ap()[:, :], h_fold[:])

                # -------- h_loc update (from local fp16 slab) ----
                if not last:
                    bnl = OC.tile([128, SLOTS], F16, tag="bigs")
                    nc.scalar.activation(
                        bnl[0:64, :], o_slab[0:64, :], AF.Relu,
                        scale=sc[0:64, 0:1], bias=sc[0:64, 1:2],
                    )
                    if l == 0:
                        nc.vector.tensor_copy(h_loc[:], bnl[0:64, :])
                    else:
                        nc.vector.tensor_tensor(
                            h_loc[:], h_loc[:], bnl[0:64, :], ALU.add
                        )

            # ---------------- output heads ----------------
            headW_sb = P.tile([128, 2], FP32, tag="headW_sb")
            nc.sync.dma_start(headW_sb[:], headW_d[:, :])
            headb_sb = P.tile([2, 1], FP32, tag="headb_sb")
            nc.sync.dma_start(headb_sb[:], headb_d[:, :])
            for half in range(2):
                base = 64 * half
                for j in range(HALF2 // 512):
                    hp = TRP.tile([2, 512], FP32, tag="tp")
                    nc.tensor.matmul(
                        hp[:],
                        headW_sb[base : base + 32, :],
                        h_fold[base : base + 32, j * 512 : (j + 1) * 512],
                        start=True, stop=True,
                    )
                    hs = SM.tile([2, 512], FP32, tag="hs")
                    nc.scalar.activation(hs[:], hp[:], AF.Identity, bias=headb_sb[:])
                    hs2 = SM.tile([2, 512], FP32, tag="hs2")
                    nc.scalar.activation(hs2[:], hs[:], AF.Sigmoid)
                    # row 0 = imp (linear), row 1 = pol (sigmoid)
                    col0 = half * HALF2 + j * 512
                    nc.sync.dma_start(out_d.ap()[0:1, col0 : col0 + 512], hs[0:1, :])
                    nc.sync.dma_start(out_d.ap()[1:2, col0 : col0 + 512], hs2[1:2, :])

    nc.compile()
    return nc


# ===================== host side =====================

def make_cfg(deg, ncores=8, nlayers=20, f_in=17):
    n = deg.shape[0]
    real = n // ncores
    blocks = (real + 127) // 128
    order = np.argsort(deg, kind="stable")
    Ks = np.zeros((ncores, blocks), np.int64)
    for c in range(ncores):
        dc = deg[order[c::ncores]]
        for b in range(blocks):
            blk = dc[b * 128 : (b + 1) * 128]
            Ks[c, b] = blk.max() if blk.size else 1
    K_sched = tuple(int(max(x, 1)) for x in Ks.max(axis=0))
    cfg = Cfg(ncores=ncores, blocks=blocks, real_per_core=real,
              nlayers=nlayers, K_sched=K_sched, f_in=f_in)
    return order, cfg


def host_prep(inputs, nlayers=20, ncores=8):
    """Build cfg, per-core input maps, and the slot->node mapping."""
    x = np.asarray(inputs["x"], np.float32)
    src = np.asarray(inputs["src"], np.int64)
    dst = np.asarray(inputs["dst"], np.int64)
    n = x.shape[0]
    loop = np.arange(n, dtype=np.int64)
    s_all = np.concatenate([src, loop])
    d_all = np.concatenate([dst, loop])
    deg = np.bincount(d_all, minlength=n)

    order, cfg = make_cfg(deg, ncores=ncores, nlayers=nlayers, f_in=x.shape[1])
    SLOTS = cfg.slots
    NPAD = cfg.npad
    L = nlayers

    slot_of_node = np.full(n, -1, np.int64)
    for c in range(ncores):
        nodes = order[c::ncores]
        slot_of_node[nodes] = c * SLOTS + np.arange(nodes.shape[0])
    assert (slot_of_node >= 0).all()

    s_slot = slot_of_node[s_all]
    d_slot = slot_of_node[d_all]

    Ksch = cfg.K_sched
    sumk = cfg.sumk
    koff = np.concatenate([[0], np.cumsum(Ksch)]).astype(np.int64)
    order_e = np.argsort(d_slot, kind="stable")
    ds_sorted = d_slot[order_e]
    ss_sorted = s_slot[order_e]
    starts = np.searchsorted(ds_sorted, np.arange(NPAD))
    ends = np.searchsorted(ds_sorted, np.arange(NPAD) + 1)

    idx_maps, mask_maps = [], []
    for c in range(ncores):
        mask = np.full((128, sumk), np.float32(NEG16), np.float32)
        idx_arr = np.zeros((16, 8 * sumk), np.int16)
        for b in range(cfg.blocks):
            K = int(Ksch[b])
            flat = np.zeros(128 * K, np.int16)
            for lane in range(128):
                sl = c * SLOTS + b * 128 + lane
                e0, e1 = starts[sl], ends[sl]
                kdeg = e1 - e0
                assert kdeg <= K, (kdeg, K, b)
                if kdeg:
                    flat[np.arange(kdeg) * 128 + lane] = ss_sorted[e0:e1].astype(np.int16)
                    mask[lane, koff[b] : koff[b] + kdeg] = 0.0
            blk = flat.reshape(8 * K, 16)
            idx_arr[:, 8 * koff[b] : 8 * koff[b] + 8 * K] = blk.T
        idx_maps.append(idx_arr)
        mask_maps.append(mask.astype(F16NP))

    xp = np.zeros((NPAD, 32), np.float32)
    xp[slot_of_node[np.arange(n)], : x.shape[1]] = x
    HALF2 = cfg.half2
    f_in = x.shape[1]
    xTf = np.ascontiguousarray(
        np.concatenate([xp[:HALF2, :f_in].T, xp[HALF2:, :f_in].T], axis=0)
    ).astype(F16NP)
    xloc_maps = [
        np.ascontiguousarray(xp[c * SLOTS : (c + 1) * SLOTS, :f_in].T)
        for c in range(ncores)
    ]

    def wpack(W, KC):
        out = np.zeros((64, 128), np.float32)
        kin, wout = W.shape
        out[0:kin, 0:wout] = W
        return out

    Wl_all = np.zeros((L, 64, 128), np.float32)
    Wr_all = np.zeros((L, 64, 128), np.float32)
    attR = np.zeros((L, 1, 128), np.float32)
    xrb = np.zeros((L, 128), np.float32)
    beff = np.zeros((L, 64), np.float32)
    g_all = np.zeros((L, 64), np.float32)
    be_all = np.zeros((L, 64), np.float32)

    def layer_params(l):
        if l == 0:
            return (inputs["W0l"], inputs["b0l"], inputs["W0r"], inputs["b0r"],
                    inputs["att0"], inputs["bias0"], inputs["g0"], inputs["be0"])
        if l < L - 1:
            i = l - 1
            return (inputs["Wml"][i], inputs["bml"][i], inputs["Wmr"][i],
                    inputs["bmr"][i], inputs["attm"][i], inputs["biasm"][i],
                    inputs["gm"][i], inputs["bem"][i])
        return (inputs["WLl"], inputs["bLl"], inputs["WLr"], inputs["bLr"],
                inputs["attL"], inputs["biasL"], inputs["gL"], inputs["beL"])

    for l in range(L):
        Wl, bl, Wr, br, att, bias, g, be = [
            np.asarray(a, np.float32) for a in layer_params(l)
        ]
        KC = 32 if l == 0 else 64
        w = 128 if l < L - 1 else 64
        w2 = w // 2
        Wl_all[l] = wpack(Wl, KC)
        Wr_all[l] = wpack(Wr, KC)
        attR[l, 0, 0:w] = att.reshape(w)
        xrb[l, 0:w] = bl + br
        beff[l, 0:w2] = bias + 0.5 * (bl.reshape(2, w2)[0] + bl.reshape(2, w2)[1])
        g_all[l, 0:w2] = g
        be_all[l, 0:w2] = be

    headW = np.zeros((128, 2), np.float32)
    Wimp = np.asarray(inputs["Wimp"], np.float32)
    Wpol = np.asarray(inputs["Wpol"], np.float32)
    d_out = Wimp.shape[0]
    headW[0:d_out, 0] = Wimp[:, 0]
    headW[0:d_out, 1] = Wpol[:, 0]
    headW[64 : 64 + d_out, 0] = Wimp[:, 0]
    headW[64 : 64 + d_out, 1] = Wpol[:, 0]
    headb = np.array(
        [[np.float32(np.asarray(inputs["bimp"]).reshape(-1)[0])],
         [np.float32(np.asarray(inputs["bpol"]).reshape(-1)[0])]], np.float32
    )

    shared = dict(
        xTf=xTf,
        Wl=Wl_all.astype(F16NP), Wr=Wr_all, attR=attR,
        xrb=xrb, beff=beff,
        gbn=g_all, bebn=be_all,
        ident=np.eye(128, dtype=np.float32),
        headW=headW, headb=headb,
    )
    in_maps = []
    for c in range(ncores):
        m = dict(shared)
        m["idx"] = idx_maps[c]
        m["mask"] = mask_maps[c]
        m["xloc"] = xloc_maps[c]
        in_maps.append(m)
    return cfg, in_maps, slot_of_node


# ===================== entry point =====================

LAST_EXEC_NS = None
_TRACE = bool(int(__import__("os").environ.get("GNN_TRACE", "0")))
_NC_CACHE = {}


def _built(cfg):
    # memoize by the cfg fields that determine the generated program, so
    # repeated kernel() calls in one process skip the ~2s rebuild
    key = (cfg.ncores, cfg.blocks, cfg.real_per_core, cfg.nlayers,
           cfg.K_sched, cfg.f_in)
    nc = _NC_CACHE.get(key)
    if nc is None:
        nc = _NC_CACHE[key] = build_kernel(cfg)
    return nc


def kernel(**inputs):
    """Full-input GATv2 GNN on 8 trn2 NeuronCores; returns (imp, pol)."""
    global LAST_EXEC_NS
    from concourse.bass_utils import run_bass_kernel_spmd

    cfg, in_maps, slot_of_node = host_prep(inputs, nlayers=20, ncores=8)
    nc = _built(cfg)
    res = run_bass_kernel_spmd(
        nc, in_maps, core_ids=list(range(cfg.ncores)), trace=_TRACE
    )
    LAST_EXEC_NS = res.exec_time_ns
    out = np.asarray(res.results[0]["out"])
    imp = out[0][slot_of_node][:, None]
    pol = out[1][slot_of_node][:, None]
    return imp.astype(np.float32), pol.astype(np.float32)

